# revision 5
# baseline (speedup 1.0000x reference)
"""Additive attention (B=16, Q=128, K=1024, D=256, H=64) on 8 trn2 NeuronCores.

scores[b,q,k] = sum_h Wv[h] * tanh(qproj[b,q,h] + kproj[b,k,h]); softmax over
valid k only; out = attn @ values.

v2 design (QCH=32): a work unit is (batch, 32-row q-chunk).  64 units sorted
by valid_len desc -> 8 slots of 8 units; slot j runs SPMD on the 8 cores with
compile-time K extent ks_j = slot max valid_len.  Per (core, slot):
  - PE kproj with duplicated weights wk2 [D,128] -> psum [128, ks] (row
    64*par+h = kproj[.,h] twice); ONE DVE copy -> kp bf16 sbuf.
  - PE qproj -> psum [128, PACKS] (par halves at partition 0/64), one DVE
    copy -> qp f32.
  - DVE tensor_scalar_add (bf16 4x): feat[:, p, :] = kp + qp[:, p]
  - ACT tanh over [128, GS*cw] chunks (the bound: 1 col/cycle @1.2GHz)
  - PE score accumulation with Wv embedded in wvs lhsT -> psum sc [32, ks]
  - ACT exp straight from psum -> attn bf16 sbuf [32, ks] (no max-sub;
    |score| <= sum|Wv|, host-checked)
  - DMA xbar transpose [32, kcp] -> aT [128, kc, 32] bf16 (no PE transpose,
    no DVE mask: rows >= own valid_len are host-zeroed in values_aug, so
    garbage attn columns multiply zero rows; col 256 of values_aug is the
    ones column giving the softmax denominator via the AV matmul)
  - PE AV: aT chunks @ values_aug -> [32, 258] psum
  - DVE: out = av[:, :256] * reciprocal(av[:, 256]); store via gpsimd queue.
"""

import sys

for _p in ("/opt/trn_rl_repo",):
    if _p not in sys.path:
        sys.path.append(_p)

import numpy as np
import ml_dtypes

import concourse.bass as bass  # noqa: F401
import concourse.tile as tile
from concourse import bacc, mybir
from concourse.bass_utils import run_bass_kernel_spmd

F32 = mybir.dt.float32
BF16 = mybir.dt.bfloat16
BF = ml_dtypes.bfloat16

B, Q, K, D, H, V = 16, 128, 1024, 256, 64, 256
VW = 258          # 256 values + ones column + pad
NCORES = 8
import os as _os
QCH = int(_os.environ.get("AK_QCH", "32"))
PACKS = QCH // 2
GS = min(8, PACKS)                       # packs per tanh group
CW = int(_os.environ.get("AK_CW", "512"))  # k chunk width
FEAT_BUFS = int(_os.environ.get("AK_FEAT_BUFS", "3"))
STORE_ENG = _os.environ.get("AK_STORE", "gpsimd")
NSLOTS = (B * (Q // QCH)) // NCORES

_cache = {}


def _build(ks_list, exp_shift):
    nc = bacc.Bacc("TRN2", target_bir_lowering=False, debug=False,
                   num_devices=NCORES)
    kcs = [(ks + 127) // 128 for ks in ks_list]

    kT_d = [nc.dram_tensor(f"kT{j}", [D, ks], BF16, kind="ExternalInput")
            for j, ks in enumerate(ks_list)]
    vA_d = [nc.dram_tensor(f"vA{j}", [kc * 128, VW], BF16, kind="ExternalInput")
            for j, kc in enumerate(kcs)]
    qT_d = [nc.dram_tensor(f"qT{j}", [D, QCH], F32, kind="ExternalInput")
            for j in range(NSLOTS)]
    wk2_d = nc.dram_tensor("wk2", [D, 128], BF16, kind="ExternalInput")
    wqT_d = nc.dram_tensor("wqT", [D, H], F32, kind="ExternalInput")
    wvs_d = nc.dram_tensor("wvs", [128, PACKS * QCH], BF16, kind="ExternalInput")
    out_d = nc.dram_tensor("out", [NSLOTS, QCH, V], F32, kind="ExternalOutput")

    with tile.TileContext(nc) as tc:
        with (
            tc.tile_pool(name="const", bufs=1) as const,
            tc.tile_pool(name="sb_k", bufs=2) as sb_k,
            tc.tile_pool(name="sb_v", bufs=2) as sb_v,
            tc.tile_pool(name="sb_q", bufs=2) as sb_q,
            tc.tile_pool(name="sb_kp", bufs=2) as sb_kp,
            tc.tile_pool(name="sb_feat", bufs=FEAT_BUFS) as sb_feat,
            tc.tile_pool(name="sb_tanh", bufs=FEAT_BUFS) as sb_tanh,
            tc.tile_pool(name="sb_attn", bufs=2) as sb_attn,
            tc.tile_pool(name="sb_aT", bufs=2) as sb_aT,
            tc.tile_pool(name="sb_out", bufs=2) as sb_out,
            tc.tile_pool(name="ps_kp", bufs=2, space="PSUM") as ps_kp,
            tc.tile_pool(name="ps_sc", bufs=2, space="PSUM") as ps_sc,
            tc.tile_pool(name="ps_av", bufs=2, space="PSUM") as ps_av,
        ):
            def load_slot(j, split_kt=False):
                ks, kc = ks_list[j], kcs[j]
                qt = sb_q.tile([128, 2, QCH], F32, tag="qt", name=f"qt{j}")
                nc.sync.dma_start(out=qt, in_=qT_d[j].ap().rearrange(
                    "(c p) q -> p c q", p=128))
                kt = sb_k.tile([128, 2, ks], BF16, tag="kt", name=f"kt{j}")
                ktsrc = kT_d[j].ap().rearrange("(c p) k -> p c k", p=128)
                if split_kt and ks > CW:
                    nc.sync.dma_start(out=kt[:, :, :CW], in_=ktsrc[:, :, :CW])
                    nc.sync.dma_start(out=kt[:, :, CW:], in_=ktsrc[:, :, CW:])
                else:
                    nc.sync.dma_start(out=kt, in_=ktsrc)
                vt = sb_v.tile([128, kc, VW], BF16, tag="vt", name=f"vt{j}")
                nc.sync.dma_start(out=vt, in_=vA_d[j].ap().rearrange(
                    "(c p) v -> p c v", p=128))
                return kt, qt, vt

            preload = {0: load_slot(0, split_kt=True)}

            wk2_sb = const.tile([128, 2, 128], BF16)
            nc.sync.dma_start(out=wk2_sb, in_=wk2_d.ap().rearrange(
                "(c p) h -> p c h", p=128))
            wq_sb = const.tile([128, 2, H], F32)
            nc.sync.dma_start(out=wq_sb, in_=wqT_d.ap().rearrange(
                "(c p) h -> p c h", p=128))
            wvs_sb = const.tile([128, PACKS, QCH], BF16)
            nc.sync.dma_start(out=wvs_sb, in_=wvs_d.ap().rearrange(
                "p (k m) -> p k m", k=PACKS))
            warm = const.tile([128, 2], F32)
            nc.vector.memset(warm, 0.0)
            nc.scalar.activation(warm[:, 1:2], warm[:, 0:1],
                                 mybir.ActivationFunctionType.Tanh)

            store_eng = nc.gpsimd if STORE_ENG == "gpsimd" else nc.sync

            for j in range(NSLOTS):
                ks, kc = ks_list[j], kcs[j]
                if j == 0 and CW < ks <= 640:
                    sc_chunks = [(0, CW)] + [
                        (s, min(CW, ks - s)) for s in range(CW, ks, CW)]
                else:
                    sc_chunks = [(s, min(CW, ks - s)) for s in range(0, ks, CW)]

                kt, qt, vt = preload.pop(j) if j in preload else load_slot(j)

                # ---- qproj packed -> qp_sb f32 [128, PACKS]
                qp_sb = sb_q.tile([128, PACKS], F32, tag="qp")
                for par in (0, 1):
                    qp_ps = ps_kp.tile([64, PACKS], F32, tag="kp",
                                       name=f"qp_ps{j}_{par}")
                    for dc in (0, 1):
                        nc.tensor.matmul(
                            qp_ps[:, :],
                            wq_sb[:, dc, :],
                            qt[:, dc, par::2],
                            start=(dc == 0), stop=(dc == 1))
                    nc.vector.tensor_copy(
                        qp_sb[64 * par:64 * par + 64, :], qp_ps)

                # ---- kprojT duplicated rows -> psum [128, cw] -> kp bf16
                kp = sb_kp.tile([128, ks], BF16, tag="kp")
                for s0, cw in sc_chunks:
                    kp_ps = ps_kp.tile([128, cw], F32, tag="kp")
                    for dc in (0, 1):
                        nc.tensor.matmul(
                            kp_ps[:, :],
                            wk2_sb[:, dc, :],
                            kt[:, dc, s0:s0 + cw],
                            start=(dc == 0), stop=(dc == 1))
                    nc.vector.tensor_copy(kp[:, s0:s0 + cw], kp_ps)

                # ---- features -> tanh -> scores  (per chunk, groups of GS)
                sc_ps = ps_sc.tile([QCH, ks], F32, tag="sc", name=f"sc{j}")
                ngroups = PACKS // GS
                for ci, (s0, cw) in enumerate(sc_chunks):
                    for g in range(ngroups):
                        feat = sb_feat.tile([128, GS, cw], BF16,
                                            tag="feat",
                                            name=f"feat{j}_{g}_{ci}")
                        tanhg = sb_tanh.tile([128, GS, cw], BF16,
                                             tag="tanh",
                                             name=f"tanh{j}_{g}_{ci}")
                        for p8 in range(GS):
                            p = g * GS + p8
                            nc.vector.tensor_scalar_add(
                                feat[:, p8, :], kp[:, s0:s0 + cw],
                                qp_sb[:, p:p + 1])
                        nc.scalar.activation(
                            tanhg[:, :, :], feat[:, :, :],
                            mybir.ActivationFunctionType.Tanh)
                        for p8 in range(GS):
                            p = g * GS + p8
                            nc.tensor.matmul(
                                sc_ps[:, s0:s0 + cw],
                                wvs_sb[:, p, :],
                                tanhg[:, p8, :],
                                start=(p == 0), stop=(p == PACKS - 1))

                # ---- exp from psum -> attn bf16 sbuf
                attn = sb_attn.tile([QCH, kc * 128], BF16, tag="attn")
                if ks < kc * 128:
                    nc.gpsimd.memset(attn[:, ks:], 0.0)
                nc.scalar.activation(
                    attn[:, :ks], sc_ps[:, :],
                    mybir.ActivationFunctionType.Exp,
                    bias=-exp_shift)

                # ---- DMA xbar transpose -> aT [128, kc, QCH]
                # (attn cols ks..kc*128 are stale garbage but every aT row
                #  >= own valid_len hits a host-zeroed values_aug row)
                aT = sb_aT.tile([128, kc, QCH], BF16, tag="aT")
                nc.sync.dma_start(out=aT, in_=attn, transpose=True)

                # ---- AV
                av_ps = ps_av.tile([QCH, VW], F32, tag="av")
                for t in range(kc):
                    nc.tensor.matmul(
                        av_ps[:, :], aT[:, t, :], vt[:, t, :],
                        start=(t == 0), stop=(t == kc - 1))

                # ---- normalize + store
                rcp = sb_out.tile([QCH, 1], F32, tag="rcp")
                nc.vector.reciprocal(rcp, av_ps[:, V:V + 1])
                outt = sb_out.tile([QCH, V], F32, tag="out")
                nc.vector.tensor_scalar_mul(outt, av_ps[:, 0:V], rcp)
                store_eng.dma_start(out=out_d.ap()[j], in_=outt)

    nc.compile()
    return nc


def _prep(queries, keys, values, valid_lens, Wq, Wk, Wv):
    vl = [int(x) for x in np.asarray(valid_lens).reshape(-1)]
    assert len(vl) == B
    units = sorted(
        [(vl[b], b, h) for b in range(B) for h in range(Q // QCH)],
        key=lambda u: -u[0])
    ks_list = [units[NCORES * j][0] for j in range(NSLOTS)]
    kcs = [(ks + 127) // 128 for ks in ks_list]

    qT = np.ascontiguousarray(np.transpose(np.asarray(queries, np.float32),
                                           (0, 2, 1)))          # [B, D, Q]
    kT = np.ascontiguousarray(np.transpose(np.asarray(keys, BF), (0, 2, 1)))
    va = np.zeros((B, K, VW), BF)
    va[:, :, :V] = np.asarray(values, BF)
    va[:, :, V] = BF(1.0)

    wkT = np.ascontiguousarray(np.asarray(Wk, BF).T)             # [D, H]
    wk2 = np.ascontiguousarray(np.concatenate([wkT, wkT], axis=1))  # [D, 128]
    wqT = np.ascontiguousarray(np.asarray(Wq, np.float32).T)     # [D, H]
    wv = np.asarray(Wv, np.float32).reshape(-1)                  # [H]
    bound = float(np.abs(wv).sum())
    exp_shift = max(0.0, bound - 30.0)

    wvs = np.zeros((128, PACKS * QCH), BF)
    wvb = wv.astype(BF)
    for p in range(PACKS):
        for par in (0, 1):
            wvs[64 * par:64 * par + 64, p * QCH + 2 * p + par] = wvb

    in_maps = []
    assignment = []
    for c in range(NCORES):
        m = {"wk2": wk2, "wqT": wqT, "wvs": wvs}
        amap = []
        for j in range(NSLOTS):
            myvl, b, h = units[NCORES * j + c]
            ks, kc = ks_list[j], kcs[j]
            amap.append((b, h))
            m[f"kT{j}"] = np.ascontiguousarray(kT[b, :, :ks])
            vslice = va[b, :kc * 128, :].copy()
            vslice[myvl:, :] = 0
            m[f"vA{j}"] = np.ascontiguousarray(vslice)
            m[f"qT{j}"] = np.ascontiguousarray(
                qT[b, :, h * QCH:(h + 1) * QCH])
        in_maps.append(m)
        assignment.append(amap)
    return tuple(ks_list), exp_shift, in_maps, assignment


def kernel(queries, keys, values, valid_lens, Wq, Wk, Wv):
    ks_list, exp_shift, in_maps, assignment = _prep(
        queries, keys, values, valid_lens, Wq, Wk, Wv)
    key = (ks_list, round(exp_shift, 3))
    if key not in _cache:
        _cache[key] = _build(list(ks_list), exp_shift)
    nc = _cache[key]
    res = run_bass_kernel_spmd(nc, in_maps, list(range(NCORES)))
    out = np.zeros((B, Q, V), np.float32)
    for c in range(NCORES):
        o = res.results[c]["out"]           # [NSLOTS, QCH, V]
        for j, (b, h) in enumerate(assignment[c]):
            out[b, h * QCH:(h + 1) * QCH, :] = o[j]
    return out


if __name__ == "__main__":
    from concourse.bass_interp import CoreSim

    rng = np.random.default_rng(0)
    queries = rng.standard_normal((B, Q, D), np.float32)
    keys = rng.standard_normal((B, K, D), np.float32)
    values = rng.standard_normal((B, K, V), np.float32)
    valid_lens = rng.integers(1, K + 1, (B,)).astype(np.int64)
    Wq = (rng.standard_normal((H, D), np.float32) / np.sqrt(D)).astype(np.float32)
    Wk = (rng.standard_normal((H, D), np.float32) / np.sqrt(D)).astype(np.float32)
    Wv = (rng.standard_normal((1, H), np.float32) / np.sqrt(H)).astype(np.float32)

    ks_list, exp_shift, in_maps, assignment = _prep(
        queries, keys, values, valid_lens, Wq, Wk, Wv)
    print("ks_list:", ks_list, "exp_shift:", exp_shift)
    nc = _build(list(ks_list), exp_shift)
    print("built+compiled")

    sim = CoreSim(nc, trace=False)
    for name, arr in in_maps[0].items():
        sim.tensor(name)[:] = arr
    sim.simulate()
    got = np.array(sim.tensor("out"))

    q = queries @ Wq.T
    k = keys @ Wk.T
    worst = 0.0
    for j, (b, h) in enumerate(assignment[0]):
        feats = np.tanh(q[b, h * QCH:(h + 1) * QCH, None, :] + k[b, None, :, :])
        scores = feats @ Wv[0]
        vlb = int(valid_lens[b])
        scores[:, vlb:] = -1e6
        e = np.exp(scores - scores.max(-1, keepdims=True))
        attn = e / e.sum(-1, keepdims=True)
        exp_out = attn @ values[b]
        err = np.abs(got[j] - exp_out)
        rel = err.max() / np.abs(exp_out).max()
        worst = max(worst, rel)
        print(f"slot {j} (b={b},h={h}, vl={vlb}): absmax-rel err {rel:.3e}")
    print("worst:", worst)
